# revision 1
# baseline (speedup 1.0000x reference)
"""Trainium2 Bass kernel for nn_BasicTransformerBlock (sparse attention video block).

Strategy (8 NeuronCores, SPMD):
  Phase A (frame-parallel): core i owns frames (2i, 2i+1). LN1 + q/k/v
  projections + sparse-causal attention (keys = frame 0 + previous frame) +
  output projection + residual + LN3 + GEGLU FF + residual.
  Reshard: PE-transpose h2 to token-major, AllToAll so each core ends up
  with all 16 frames of a 128-wide spatial-token slice, PE-transpose back
  into feature-major seq-major layout.
  Phase B (spatial-parallel): LNt + temporal attention over the 16 frames of
  each spatial token (block-diagonal masked 128x128 scores) + out proj +
  residual.

Layouts: activations are feature-major [chan on partitions, tokens on free].
LayerNorm stats use ones-matmul reductions on the TensorEngine (output rows
replicated = free partition-broadcast).  All matmuls are bf16 with fp32 PSUM;
the residual path stays fp32.  SBUF is tight, so projections / attention
intermediates are staged in DRAM and streamed; pools are phase-scoped.
LN affine params are folded into the projection weights on the host; the
attn1 head-merge quirk (channel-major interleave) is folded into o1w's row
permutation on the host.
"""

import math

import numpy as np
import ml_dtypes

import concourse.bass as bass
import concourse.bacc as bacc
import concourse.tile as tile
from concourse import mybir
from concourse.bass_utils import run_bass_kernel_spmd
from concourse.masks import make_identity

BF16 = mybir.dt.bfloat16
F32 = mybir.dt.float32
AF = mybir.ActivationFunctionType
OP = mybir.AluOpType

H, DH, C, F, D = 8, 80, 640, 16, 1024
INNER = 4 * C            # 2560
NI = INNER // 128        # 20
NCORES = 8
KT = C // 128            # 5 feature k-tiles
NQ = 2 * D               # 2048 own tokens / core
NKV = 3 * D              # 3072 kv-context tokens / core
NU = 4 * D               # 4096 union tokens / core
EPS = 1e-5
RG = [list(range(NCORES))]

_BUILD_CACHE = {}


def _build(reps=1, use_cc=True):
    key = (reps, use_cc)
    if key in _BUILD_CACHE:
        return _BUILD_CACHE[key]
    nc = bacc.Bacc("TRN2", target_bir_lowering=False, debug=False,
                   num_devices=NCORES)

    def din(name, shape, dt):
        return nc.dram_tensor(name, shape, dt, kind="ExternalInput").ap()

    xub_d = din("xub", [KT, 128, NU], BF16)     # union x, feature-major bf16
    xuq_d = din("xuq", [KT, 128, NQ], F32)      # raw own x (residual), fm f32
    wq_d = din("wq", [KT, 128, C], BF16)
    wk_d = din("wk", [KT, 128, C], BF16)
    wv_d = din("wv", [KT, 128, C], BF16)
    wo1_d = din("wo1", [H, 81, C], BF16)        # per-head o-proj (row 80 = 0)
    wf1_d = din("wf1", [KT, 128, 2 * INNER], BF16)
    wf2_d = din("wf2", [NI, 128, C], BF16)
    wqt_d = din("wqt", [KT, 128, C], BF16)
    wkt_d = din("wkt", [KT, 128, C], BF16)
    wvt_d = din("wvt", [KT, 128, C], BF16)
    wot_d = din("wot", [H, 81, C], BF16)
    bq_d = din("bq", [C], F32)
    bk_d = din("bk", [C], F32)
    bv_d = din("bv", [C], F32)
    bo1_d = din("bo1", [C], F32)
    bf1_d = din("bf1", [2 * INNER], F32)
    bf2_d = din("bf2", [C], F32)
    bqt_d = din("bqt", [C], F32)
    bkt_d = din("bkt", [C], F32)
    bvt_d = din("bvt", [C], F32)
    bot_d = din("bot", [C], F32)
    mask_d = din("mask", [128, 128], BF16)
    out_d = nc.dram_tensor("out", [KT, 128, NQ], F32,
                           kind="ExternalOutput").ap()

    with tile.TileContext(nc) as tc:
        with tc.tile_pool(name="const", bufs=1) as constp, \
             tc.tile_pool(name="dram", bufs=1, space="DRAM") as dramp, \
             tc.tile_pool(name="ps512", bufs=2, space="PSUM") as ps512, \
             tc.tile_pool(name="pspv", bufs=1, space="PSUM") as pspv:

            # ---------------- global constants (~4KB/partition) -------------
            ident = constp.tile([128, 128], F32)
            make_identity(nc, ident)
            ones_b = constp.tile([128, 128], BF16)
            nc.vector.memset(ones_b, 1.0)
            epst = constp.tile([128, 1], F32)
            nc.vector.memset(epst, EPS)
            mask_t = constp.tile([128, 128], BF16)
            nc.sync.dma_start(out=mask_t, in_=mask_d[:, :])

            def bias_tile(b_d, ncols):
                t = constp.tile([128, ncols], F32, tag=f"b_{b_d.tensor.name}",
                                name=f"b_{b_d.tensor.name}")
                nc.sync.dma_start(out=t, in_=b_d.rearrange("(m p) -> p m", p=128))
                return t

            def head_bias_tile(b_d):
                # [80, H]: column h = bias[h*80:(h+1)*80]
                t = constp.tile([80, H], F32, tag=f"hb_{b_d.tensor.name}",
                                name=f"hb_{b_d.tensor.name}")
                nc.sync.dma_start(out=t, in_=b_d.rearrange("(h p) -> p h", p=DH))
                return t

            bq_t = bias_tile(bq_d, KT)
            bk_t = bias_tile(bk_d, KT)
            bqth_t = head_bias_tile(bqt_d)
            bkth_t = head_bias_tile(bkt_d)
            bo1_t = bias_tile(bo1_d, KT)
            bf1_t = bias_tile(bf1_d, 2 * INNER // 128)
            bf2_t = bias_tile(bf2_d, KT)
            bot_t = bias_tile(bot_d, KT)
            bvbc = constp.tile([128, C], F32)
            nc.sync.dma_start(out=bvbc, in_=bass.AP(
                tensor=bv_d.tensor, offset=bv_d.offset,
                ap=[[0, 128]] + list(bv_d.ap)))
            bvtbc = constp.tile([128, C], F32)
            nc.sync.dma_start(out=bvtbc, in_=bass.AP(
                tensor=bvt_d.tensor, offset=bvt_d.offset,
                ap=[[0, 128]] + list(bvt_d.ap)))

            def ln_stats(pool, wkp, load_stripe, ntok, tag):
                """load_stripe(kt, sl) -> bf16 [128, 512] stripe AP.
                Returns (M, R) bf16 [128, ntok] (per-token mean / rstd,
                replicated across partitions)."""
                M = pool.tile([128, ntok], BF16, tag=f"M_{tag}", name=f"M_{tag}")
                R = pool.tile([128, ntok], BF16, tag=f"R_{tag}", name=f"R_{tag}")
                for ch in range(ntok // 512):
                    sl = slice(ch * 512, (ch + 1) * 512)
                    stripes = [load_stripe(kt, sl) for kt in range(KT)]
                    ps_s = ps512.tile([128, 512], F32, tag="ps")
                    for kt in range(KT):
                        nc.tensor.matmul(ps_s, ones_b, stripes[kt],
                                         start=(kt == 0), stop=(kt == KT - 1))
                    ps_q = ps512.tile([128, 512], F32, tag="ps")
                    for kt in range(KT):
                        sq = wkp.tile([128, 512], BF16, tag="sq")
                        nc.vector.tensor_mul(sq, stripes[kt], stripes[kt])
                        nc.tensor.matmul(ps_q, ones_b, sq,
                                         start=(kt == 0), stop=(kt == KT - 1))
                    Mf = wkp.tile([128, 512], F32, tag="Mf")
                    nc.scalar.activation(out=Mf, in_=ps_s, func=AF.Identity,
                                         scale=1.0 / C)
                    nc.vector.tensor_copy(M[:, sl], Mf)
                    msq = wkp.tile([128, 512], F32, tag="msq")
                    nc.vector.tensor_mul(msq, Mf, Mf)
                    var = wkp.tile([128, 512], F32, tag="var")
                    nc.vector.scalar_tensor_tensor(
                        out=var, in0=ps_q, scalar=1.0 / C, in1=msq,
                        op0=OP.mult, op1=OP.subtract)
                    sd = wkp.tile([128, 512], F32, tag="sd")
                    nc.scalar.activation(out=sd, in_=var, func=AF.Sqrt,
                                         bias=epst)
                    with nc.allow_low_precision(reason="rstd in bf16 is fine "
                                                "for standardization"):
                        nc.vector.reciprocal(out=R[:, sl], in_=sd)
                return M, R

            def emit(it):
                # DRAM staging (fixed tags -> recycled across reps)
                kh_d = dramp.tile([C, NKV], BF16, tag="kh_d", name="kh_d")
                qh_d = dramp.tile([C, NQ], BF16, tag="qh_d", name="qh_d")
                v3_d = dramp.tile([NKV // 128, 128, H, 81], BF16, tag="v3_d",
                                  name="v3_d")
                h1_d = dramp.tile([KT, 128, NQ], F32, tag="h1_d", name="h1_d")
                a2a_in = dramp.tile([NCORES, 256, C], F32, tag="a2a_in",
                                    name="a2a_in")
                a2a_out = dramp.tile([NCORES, 256, C], F32, tag="a2a_out",
                                     name="a2a_out")

                # ============ Phase 1: LN1 stats + q/k/v projections ========
                with tc.tile_pool(name="p1", bufs=1) as p1, \
                     tc.tile_pool(name="p1w", bufs=2) as p1w, \
                     tc.tile_pool(name="p1e", bufs=3) as p1e:
                    wq_t = p1.tile([128, KT, C], BF16, tag="wq")
                    nc.sync.dma_start(out=wq_t, in_=wq_d.rearrange("k p c -> p k c"))
                    wk_t = p1.tile([128, KT, C], BF16, tag="wk")
                    nc.sync.dma_start(out=wk_t, in_=wk_d.rearrange("k p c -> p k c"))
                    wv_t = p1.tile([128, KT, C], BF16, tag="wv")
                    nc.sync.dma_start(out=wv_t, in_=wv_d.rearrange("k p c -> p k c"))

                    def load_xub(kt, sl):
                        t = p1w.tile([128, 512], BF16, tag=f"xs{kt}",
                                     name=f"xs{kt}")
                        nc.sync.dma_start(out=t, in_=xub_d[kt, :, sl])
                        return t

                    M1, R1 = ln_stats(p1, p1w, load_xub, NU, "ln1")

                    for ch in range(NU // 512):
                        sl = slice(ch * 512, (ch + 1) * 512)
                        xh = []
                        for kt in range(KT):
                            xs = load_xub(kt, sl)
                            tmp = p1w.tile([128, 512], F32, tag="stdtmp")
                            nc.vector.tensor_sub(tmp, xs, M1[:, sl])
                            t = p1w.tile([128, 512], BF16, tag=f"xh{kt}",
                                         name=f"xh{kt}")
                            nc.vector.tensor_mul(t, tmp, R1[:, sl])
                            xh.append(t)
                        if ch < NKV // 512:          # kv-range: k and v proj
                            for m in range(KT):
                                msl = slice(m * 128, (m + 1) * 128)
                                ps = ps512.tile([128, 512], F32, tag="ps")
                                for kt in range(KT):
                                    nc.tensor.matmul(ps, wk_t[:, kt, msl], xh[kt],
                                                     start=(kt == 0),
                                                     stop=(kt == KT - 1))
                                ke = p1e.tile([128, 512], BF16, tag="ke")
                                nc.scalar.activation(out=ke, in_=ps,
                                                     func=AF.Identity,
                                                     bias=bk_t[:, m:m + 1])
                                nc.sync.dma_start(out=kh_d[msl, sl], in_=ke)
                            ve = p1e.tile([128, 4, H, 81], BF16, tag="ve")
                            nc.vector.memset(ve[:, :, :, 80], 1.0)
                            for tw in range(4):
                                tsl = slice(tw * 128, (tw + 1) * 128)
                                for hf in range(2):
                                    csl = slice(hf * 320, (hf + 1) * 320)
                                    ps = ps512.tile([128, 320], F32, tag="ps")
                                    for kt in range(KT):
                                        nc.tensor.matmul(ps, xh[kt][:, tsl],
                                                         wv_t[:, kt, csl],
                                                         start=(kt == 0),
                                                         stop=(kt == KT - 1))
                                    nc.vector.tensor_add(
                                        ve[:, tw, hf * 4:(hf + 1) * 4, 0:80],
                                        ps.rearrange("p (h c) -> p h c", c=DH),
                                        bvbc[:, csl].rearrange(
                                            "p (h c) -> p h c", c=DH))
                            nc.sync.dma_start(
                                out=v3_d[ch * 4:(ch + 1) * 4].rearrange(
                                    "t p h c -> p t h c"),
                                in_=ve)
                        if ch >= (NU - NQ) // 512:   # q-range
                            qsl = slice(ch * 512 - (NU - NQ),
                                        (ch + 1) * 512 - (NU - NQ))
                            for m in range(KT):
                                msl = slice(m * 128, (m + 1) * 128)
                                ps = ps512.tile([128, 512], F32, tag="ps")
                                for kt in range(KT):
                                    nc.tensor.matmul(ps, wq_t[:, kt, msl], xh[kt],
                                                     start=(kt == 0),
                                                     stop=(kt == KT - 1))
                                qe = p1e.tile([128, 512], BF16, tag="qe")
                                nc.scalar.activation(out=qe, in_=ps,
                                                     func=AF.Identity,
                                                     bias=bq_t[:, m:m + 1])
                                nc.sync.dma_start(out=qh_d[msl, qsl], in_=qe)

                # ============ Phase 2: sparse-causal attention ============
                with tc.tile_pool(name="p2", bufs=1) as p2, \
                     tc.tile_pool(name="p2s", bufs=2) as p2s, \
                     tc.tile_pool(name="p2w", bufs=3) as p2w, \
                     tc.tile_pool(name="p2d", bufs=9) as p2d, \
                     tc.tile_pool(name="psc2", bufs=2, space="PSUM") as psc2:
                    wo1_t = []
                    for h in range(H):
                        t = p2.tile([81, C], BF16, tag=f"wo1_{h}",
                                    name=f"wo1_{h}")
                        nc.sync.dma_start(out=t, in_=wo1_d[h])
                        wo1_t.append(t)
                    for fi in range(2):
                        attD = []
                        for h in range(H):
                            hrow = slice(h * DH, (h + 1) * DH)
                            khs = p2s.tile([80, 2048], BF16, tag="khs")
                            nc.sync.dma_start(out=khs[:, 0:1024],
                                              in_=kh_d[hrow, 0:1024])
                            nc.sync.dma_start(
                                out=khs[:, 1024:2048],
                                in_=kh_d[hrow, (1 + fi) * 1024:(2 + fi) * 1024])
                            v3s = p2s.tile([128, 16, 81], BF16, tag="v3s")
                            nc.sync.dma_start(
                                out=v3s[:, 0:8, :],
                                in_=v3_d[0:8, :, h, :].rearrange(
                                    "t p c -> p t c"))
                            nc.sync.dma_start(
                                out=v3s[:, 8:16, :],
                                in_=v3_d[8 * (1 + fi):8 * (2 + fi), :, h, :]
                                .rearrange("t p c -> p t c"))
                            qs = p2s.tile([80, 1024], BF16, tag="qs")
                            nc.sync.dma_start(
                                out=qs, in_=qh_d[hrow, fi * D:(fi + 1) * D])
                            attP = p2w.tile([81, D], F32, tag="attP", bufs=2)
                            pv = pspv.tile([81, D], F32, tag="pv")
                            for ktile in range(16):
                                ksl = slice(ktile * 128, (ktile + 1) * 128)
                                sc = psc2.tile([128, D], F32, tag="sc2")
                                nc.tensor.matmul(sc[:, 0:512], khs[:, ksl],
                                                 qs[:, 0:512],
                                                 start=True, stop=True)
                                nc.tensor.matmul(sc[:, 512:1024], khs[:, ksl],
                                                 qs[:, 512:1024],
                                                 start=True, stop=True)
                                P = p2w.tile([128, D], BF16, tag="P")
                                nc.scalar.activation(out=P, in_=sc, func=AF.Exp)
                                nc.tensor.matmul(pv[:, 0:512],
                                                 v3s[:, ktile, :], P[:, 0:512],
                                                 start=(ktile == 0),
                                                 stop=(ktile == 15))
                                nc.tensor.matmul(pv[:, 512:1024],
                                                 v3s[:, ktile, :],
                                                 P[:, 512:1024],
                                                 start=(ktile == 0),
                                                 stop=(ktile == 15))
                            nc.vector.tensor_copy(attP, pv)
                            dnm0 = p2w.tile([1, D], F32, tag="dnm0", bufs=2)
                            nc.sync.dma_start(out=dnm0, in_=attP[80:81, :])
                            nc.vector.reciprocal(out=dnm0, in_=dnm0)
                            attB = p2w.tile([80, D], F32, tag="attB", bufs=2)
                            nc.gpsimd.partition_broadcast(attB, dnm0[0:1, :],
                                                          channels=80)
                            aD = p2d.tile([81, D], BF16, tag="attD", name="attD")
                            nc.vector.memset(aD, 0.0)
                            nc.vector.tensor_mul(aD[0:80, :], attP[0:80, :],
                                                 attB)
                            attD.append(aD)
                        for m in range(KT):
                            xuqs = p2w.tile([128, D], F32, tag="xuqs")
                            nc.sync.dma_start(out=xuqs,
                                              in_=xuq_d[m, :, fi * D:(fi + 1) * D])
                            for qc in range(2):
                                qsl = slice(qc * 512, (qc + 1) * 512)
                                ps = ps512.tile([128, 512], F32, tag="ps")
                                for h in range(H):
                                    nc.tensor.matmul(
                                        ps, wo1_t[h][:, m * 128:(m + 1) * 128],
                                        attD[h][:, qsl],
                                        start=(h == 0), stop=(h == H - 1))
                                h1e = p2w.tile([128, 512], F32, tag="h1e")
                                nc.vector.scalar_tensor_tensor(
                                    out=h1e, in0=ps, scalar=bo1_t[:, m:m + 1],
                                    in1=xuqs[:, qsl], op0=OP.add, op1=OP.add)
                                nc.sync.dma_start(
                                    out=h1_d[m, :, fi * D + qc * 512:
                                             fi * D + (qc + 1) * 512],
                                    in_=h1e)

                # ============ Phase 3: LN3 + GEGLU FF + transpose ============
                with tc.tile_pool(name="p3", bufs=1) as p3, \
                     tc.tile_pool(name="p3w", bufs=2) as p3w, \
                     tc.tile_pool(name="p3e", bufs=3) as p3e, \
                     tc.tile_pool(name="p3ff", bufs=2, space="PSUM") as p3ff, \
                     tc.tile_pool(name="pstr", bufs=2, space="PSUM") as pstr:
                    def load_h1b(kt, sl):
                        hs = p3w.tile([128, 512], F32, tag=f"hs{kt}",
                                      name=f"hs{kt}")
                        nc.sync.dma_start(out=hs, in_=h1_d[kt, :, sl])
                        hb = p3w.tile([128, 512], BF16, tag="hb")
                        nc.scalar.activation(out=hb, in_=hs, func=AF.Copy)
                        return hb

                    M3, R3 = ln_stats(p3, p3w, load_h1b, NQ, "ln3")
                    xh3 = []
                    for kt in range(KT):
                        t = p3.tile([128, NQ], BF16, tag=f"xh3_{kt}",
                                    name=f"xh3_{kt}")
                        xh3.append(t)
                    for ch in range(NQ // 512):
                        sl = slice(ch * 512, (ch + 1) * 512)
                        for kt in range(KT):
                            hs = p3w.tile([128, 512], F32, tag=f"hs{kt}",
                                          name=f"hs{kt}")
                            nc.sync.dma_start(out=hs, in_=h1_d[kt, :, sl])
                            tmp = p3w.tile([128, 512], F32, tag="stdtmp3")
                            nc.vector.tensor_sub(tmp, hs, M3[:, sl])
                            nc.vector.tensor_mul(xh3[kt][:, sl], tmp, R3[:, sl])
                    h2 = []
                    for kt in range(KT):
                        t = p3.tile([128, NQ], F32, tag=f"h2_{kt}",
                                    name=f"h2_{kt}")
                        h2.append(t)
                    for qtr in range(4):
                        sl = slice(qtr * 512, (qtr + 1) * 512)
                        ffin = p3.tile([128, NI, 512], BF16, tag="ffin")
                        for j in range(NI):
                            wa = p3e.tile([128, KT, 128], BF16, tag="wf1a")
                            nc.sync.dma_start(
                                out=wa, in_=wf1_d[:, :, j * 128:(j + 1) * 128]
                                .rearrange("k p c -> p k c"))
                            wg = p3e.tile([128, KT, 128], BF16, tag="wf1g")
                            nc.sync.dma_start(
                                out=wg, in_=wf1_d[:, :, INNER + j * 128:
                                                  INNER + (j + 1) * 128]
                                .rearrange("k p c -> p k c"))
                            psa = ps512.tile([128, 512], F32, tag="ps")
                            psg = p3ff.tile([128, 512], F32, tag="ffg")
                            for kt in range(KT):
                                nc.tensor.matmul(psa, wa[:, kt, :],
                                                 xh3[kt][:, sl],
                                                 start=(kt == 0),
                                                 stop=(kt == KT - 1))
                            for kt in range(KT):
                                nc.tensor.matmul(psg, wg[:, kt, :],
                                                 xh3[kt][:, sl],
                                                 start=(kt == 0),
                                                 stop=(kt == KT - 1))
                            gg = p3e.tile([128, 512], BF16, tag="gg")
                            nc.scalar.activation(out=gg, in_=psg, func=AF.Gelu,
                                                 bias=bf1_t[:, NI + j:NI + j + 1])
                            nc.vector.scalar_tensor_tensor(
                                out=ffin[:, j, :], in0=psa,
                                scalar=bf1_t[:, j:j + 1], in1=gg,
                                op0=OP.add, op1=OP.mult)
                        for m in range(KT):
                            w2 = p3e.tile([128, NI, 128], BF16, tag="wf2")
                            nc.sync.dma_start(
                                out=w2, in_=wf2_d[:, :, m * 128:(m + 1) * 128]
                                .rearrange("k p c -> p k c"))
                            ps = ps512.tile([128, 512], F32, tag="ps")
                            for j in range(NI):
                                nc.tensor.matmul(ps, w2[:, j, :], ffin[:, j, :],
                                                 start=(j == 0),
                                                 stop=(j == NI - 1))
                            hs = p3w.tile([128, 512], F32, tag=f"hs{m}",
                                          name=f"hs{m}")
                            nc.sync.dma_start(out=hs, in_=h1_d[m, :, sl])
                            nc.vector.scalar_tensor_tensor(
                                out=h2[m][:, sl], in0=ps,
                                scalar=bf2_t[:, m:m + 1], in1=hs,
                                op0=OP.add, op1=OP.add)
                    # transpose h2 -> token-major, stage for AllToAll
                    for tt in range(NQ // 128):
                        tm = p3e.tile([128, C], F32, tag="tmrow")
                        for kt in range(KT):
                            tp = pstr.tile([128, 128], F32, tag="tr")
                            nc.tensor.transpose(
                                tp, h2[kt][:, tt * 128:(tt + 1) * 128], ident)
                            nc.vector.tensor_copy(tm[:, kt * 128:(kt + 1) * 128],
                                                  tp)
                        j = tt % 8
                        fl = tt // 8
                        nc.sync.dma_start(
                            out=a2a_in[j, fl * 128:(fl + 1) * 128, :], in_=tm)

                if use_cc:
                    nc.gpsimd.collective_compute(
                        "AllToAll", OP.bypass, replica_groups=RG,
                        ins=[a2a_in[:, :, :]], outs=[a2a_out[:, :, :]])
                else:
                    # timeline-sim stand-in with comparable DMA volume
                    nc.sync.dma_start(out=a2a_out[:, :, :], in_=a2a_in[:, :, :])

                # ============ Phase 4: temporal block ============
                with tc.tile_pool(name="p4", bufs=1) as p4, \
                     tc.tile_pool(name="p4s", bufs=2) as p4s, \
                     tc.tile_pool(name="p4w", bufs=3) as p4w, \
                     tc.tile_pool(name="p4d", bufs=8) as p4d, \
                     tc.tile_pool(name="pstr", bufs=2, space="PSUM") as pstr:
                    # ht is the temporal residual; bf16 is within tolerance
                    ht = []
                    for kt in range(KT):
                        t = p4.tile([128, NQ], BF16, tag=f"ht{kt}",
                                    name=f"ht{kt}")
                        ht.append(t)
                    for j in range(NCORES):
                        for fl in range(2):
                            rt = p4w.tile([128, C], F32, tag="rtrow")
                            nc.sync.dma_start(
                                out=rt, in_=a2a_out[j, fl * 128:(fl + 1) * 128, :])
                            fr = 2 * j + fl
                            for kt in range(KT):
                                tp = pstr.tile([128, 128], F32, tag="tr")
                                nc.tensor.transpose(
                                    tp, rt[:, kt * 128:(kt + 1) * 128], ident)
                                dst = ht[kt].rearrange("p (s f) -> p s f", f=F)
                                nc.vector.tensor_copy(dst[:, :, fr], tp)

                    Mt, Rt = ln_stats(p4, p4w, lambda kt, sl: ht[kt][:, sl],
                                      NQ, "lnt")
                    htb = []
                    for kt in range(KT):
                        t = p4.tile([128, NQ], BF16, tag=f"htb{kt}",
                                    name=f"htb{kt}")
                        htb.append(t)
                    for kt in range(KT):
                        for ch in range(NQ // 512):
                            sl = slice(ch * 512, (ch + 1) * 512)
                            tmp = p4w.tile([128, 512], F32, tag="stdtmpt")
                            nc.vector.tensor_sub(tmp, ht[kt][:, sl], Mt[:, sl])
                            nc.vector.tensor_mul(htb[kt][:, sl], tmp, Rt[:, sl])

                    wvt_t = p4.tile([128, KT, C], BF16, tag="wvt")
                    nc.sync.dma_start(out=wvt_t,
                                      in_=wvt_d.rearrange("k p c -> p k c"))

                    vt3_d = dramp.tile([NQ // 128, 128, H, 81], BF16,
                                       tag="vt3_d", name="vt3_d")
                    for tt in range(NQ // 128):
                        tsl = slice(tt * 128, (tt + 1) * 128)
                        vet = p4w.tile([128, H, 81], BF16, tag="vet")
                        nc.vector.memset(vet[:, :, 80], 1.0)
                        for hf in range(2):
                            csl = slice(hf * 320, (hf + 1) * 320)
                            ps = ps512.tile([128, 320], F32, tag="ps")
                            for kt in range(KT):
                                nc.tensor.matmul(ps, htb[kt][:, tsl],
                                                 wvt_t[:, kt, csl],
                                                 start=(kt == 0),
                                                 stop=(kt == KT - 1))
                            nc.vector.tensor_add(
                                vet[:, hf * 4:(hf + 1) * 4, 0:80],
                                ps.rearrange("p (h c) -> p h c", c=DH),
                                bvtbc[:, csl].rearrange("p (h c) -> p h c",
                                                        c=DH))
                        nc.sync.dma_start(out=vt3_d[tt], in_=vet)

                    attDt = []
                    for h in range(H):
                        wqs = p4s.tile([128, KT, DH], BF16, tag="wqs")
                        nc.sync.dma_start(
                            out=wqs, in_=wqt_d[:, :, h * DH:(h + 1) * DH]
                            .rearrange("k p c -> p k c"))
                        wks = p4s.tile([128, KT, DH], BF16, tag="wks")
                        nc.sync.dma_start(
                            out=wks, in_=wkt_d[:, :, h * DH:(h + 1) * DH]
                            .rearrange("k p c -> p k c"))
                        vts = p4s.tile([128, NQ // 128, 81], BF16, tag="vts")
                        nc.sync.dma_start(
                            out=vts, in_=vt3_d[:, :, h, :].rearrange(
                                "t p c -> p t c"))
                        qth = p4s.tile([80, NQ], BF16, tag="qth")
                        kth = p4s.tile([80, NQ], BF16, tag="kth")
                        for ch in range(NQ // 512):
                            sl = slice(ch * 512, (ch + 1) * 512)
                            ps = ps512.tile([80, 512], F32, tag="ps")
                            for kt in range(KT):
                                nc.tensor.matmul(ps, wqs[:, kt, :], htb[kt][:, sl],
                                                 start=(kt == 0),
                                                 stop=(kt == KT - 1))
                            nc.scalar.activation(out=qth[:, sl], in_=ps,
                                                 func=AF.Identity,
                                                 bias=bqth_t[:, h:h + 1])
                            ps2 = ps512.tile([80, 512], F32, tag="ps")
                            for kt in range(KT):
                                nc.tensor.matmul(ps2, wks[:, kt, :],
                                                 htb[kt][:, sl],
                                                 start=(kt == 0),
                                                 stop=(kt == KT - 1))
                            nc.scalar.activation(out=kth[:, sl], in_=ps2,
                                                 func=AF.Identity,
                                                 bias=bkth_t[:, h:h + 1])
                        attP = p4w.tile([81, NQ], F32, tag="attPt", bufs=1)
                        for tt in range(NQ // 128):
                            tsl = slice(tt * 128, (tt + 1) * 128)
                            ps_s = ps512.tile([128, 128], F32, tag="ps")
                            nc.tensor.matmul(ps_s, kth[:, tsl], qth[:, tsl],
                                             start=True, stop=True)
                            Pe = p4w.tile([128, 128], BF16, tag="Pe")
                            nc.scalar.activation(out=Pe, in_=ps_s, func=AF.Exp)
                            Pm = p4w.tile([128, 128], BF16, tag="Pm")
                            nc.vector.tensor_mul(Pm, Pe, mask_t)
                            pv = pspv.tile([81, 128], F32, tag="pv")
                            nc.tensor.matmul(pv, vts[:, tt, :], Pm,
                                             start=True, stop=True)
                            nc.vector.tensor_copy(attP[:, tsl], pv)
                        dnm0 = p4w.tile([1, NQ], F32, tag="dnm0t", bufs=2)
                        nc.sync.dma_start(out=dnm0, in_=attP[80:81, :])
                        nc.vector.reciprocal(out=dnm0, in_=dnm0)
                        attB = p4w.tile([80, NQ], F32, tag="attBt", bufs=1)
                        nc.gpsimd.partition_broadcast(attB, dnm0[0:1, :],
                                                      channels=80)
                        aD = p4d.tile([81, NQ], BF16, tag="attDt", name="attDt")
                        nc.vector.memset(aD, 0.0)
                        nc.vector.tensor_mul(aD[0:80, :], attP[0:80, :], attB)
                        attDt.append(aD)

                    for m in range(KT):
                        wos = p4s.tile([81, H, 128], BF16, tag="wos")
                        nc.sync.dma_start(
                            out=wos, in_=wot_d[:, :, m * 128:(m + 1) * 128]
                            .rearrange("h p c -> p h c"))
                        for ch in range(NQ // 512):
                            sl = slice(ch * 512, (ch + 1) * 512)
                            ps = ps512.tile([128, 512], F32, tag="ps")
                            for h in range(H):
                                nc.tensor.matmul(
                                    ps, wos[:, h, :], attDt[h][:, sl],
                                    start=(h == 0), stop=(h == H - 1))
                            oe = p4w.tile([128, 512], F32, tag="oe")
                            nc.vector.scalar_tensor_tensor(
                                out=oe, in0=ps, scalar=bot_t[:, m:m + 1],
                                in1=ht[m][:, sl], op0=OP.add, op1=OP.add)
                            nc.sync.dma_start(out=out_d[m, :, sl], in_=oe)

            for it in range(reps):
                emit(it)

    nc.compile()
    _BUILD_CACHE[key] = nc
    return nc


def _prep_inputs(hidden_states, ln1_g, ln1_b, q1w, k1w, v1w, o1w, o1b,
                 ln3_g, ln3_b, ff_w1, ff_b1, ff_w2, ff_b2,
                 lnt_g, lnt_b, qtw, ktw, vtw, otw, otb):
    """Host-side weight folding + per-core input shards."""
    bf = ml_dtypes.bfloat16
    sc = 1.0 / math.sqrt(DH)

    def fold(g, b, w):
        return g[:, None] * w, b @ w

    wq, bq = fold(ln1_g, ln1_b, q1w)
    wq, bq = wq * sc, bq * sc
    wk, bk = fold(ln1_g, ln1_b, k1w)
    wv, bv = fold(ln1_g, ln1_b, v1w)
    # o1w quirk: channel-major interleave -> padded per-head [81, C] with the
    # original row dh*H + h at padded position (h, dh); row 80 is zero
    # (multiplies the softmax-denominator row).
    wo1 = np.zeros((H, 81, C), np.float32)
    idx_dh = np.arange(DH)
    for h in range(H):
        wo1[h, 0:DH, :] = o1w[idx_dh * H + h, :]
    wf1, bf1 = fold(ln3_g, ln3_b, ff_w1)
    bf1 = bf1 + ff_b1
    wqt, bqt = fold(lnt_g, lnt_b, qtw)
    wqt, bqt = wqt * sc, bqt * sc
    wkt, bkt = fold(lnt_g, lnt_b, ktw)
    wvt, bvt = fold(lnt_g, lnt_b, vtw)
    wot = np.zeros((H, 81, C), np.float32)
    for h in range(H):
        wot[h, 0:DH, :] = otw[h * DH + idx_dh, :]

    # 8 sequences per 128-token tile; block-diag of 8 16x16 blocks
    mask = np.kron(np.eye(8, dtype=np.float32), np.ones((F, F), np.float32))

    def c(a, dt=bf):
        return np.ascontiguousarray(np.asarray(a, np.float32).astype(dt))

    shared = dict(
        wq=c(wq.reshape(KT, 128, C)), wk=c(wk.reshape(KT, 128, C)),
        wv=c(wv.reshape(KT, 128, C)), wo1=c(wo1),
        wf1=c(wf1.reshape(KT, 128, 2 * INNER)),
        wf2=c(np.asarray(ff_w2, np.float32).reshape(INNER // 128, 128, C)),
        wqt=c(wqt.reshape(KT, 128, C)), wkt=c(wkt.reshape(KT, 128, C)),
        wvt=c(wvt.reshape(KT, 128, C)), wot=c(wot),
        bq=c(bq, np.float32), bk=c(bk, np.float32), bv=c(bv, np.float32),
        bo1=c(o1b, np.float32), bf1=c(bf1, np.float32),
        bf2=c(ff_b2, np.float32),
        bqt=c(bqt, np.float32), bkt=c(bkt, np.float32),
        bvt=c(bvt, np.float32), bot=c(otb, np.float32),
        mask=c(mask),
    )

    hs = np.asarray(hidden_states, np.float32)   # [BF, D, C]
    in_maps = []
    for i in range(NCORES):
        fa, fb = 2 * i, 2 * i + 1
        fprev = max(2 * i - 1, 0)
        frames = [0, fprev, fa, fb]
        xu = hs[frames].reshape(NU, C).T          # [C, NU] feature-major
        m = dict(shared)
        m["xub"] = np.ascontiguousarray(xu.astype(bf).reshape(KT, 128, NU))
        m["xuq"] = np.ascontiguousarray(
            xu[:, NQ:].astype(np.float32).reshape(KT, 128, NQ))
        in_maps.append(m)
    return in_maps


def kernel(**inputs):
    video_length = int(np.asarray(inputs.pop("video_length")))
    assert video_length == F, f"kernel hardcodes F={F}, got {video_length}"
    in_maps = _prep_inputs(**{k: np.asarray(v) for k, v in inputs.items()})
    nc = _build(reps=1)
    res = run_bass_kernel_spmd(nc, in_maps, list(range(NCORES)))
    out = np.empty((F, D, C), np.float32)
    for i in range(NCORES):
        r = res.results[i]["out"].reshape(C, D // NCORES, F)   # [c, s, f]
        out[:, i * (D // NCORES):(i + 1) * (D // NCORES), :] = r.transpose(2, 1, 0)
    return out



# revision 32
# speedup vs baseline: 1.4805x; 1.4805x over previous
"""Trainium2 Bass kernel for nn_BasicTransformerBlock (sparse attention video block).

Strategy (8 NeuronCores, SPMD), instruction-count-minimized:
  Phase A (frame-parallel): core i owns frames (2i, 2i+1). LN1 + q/k/v
  projections + sparse-causal attention (keys = frame 0 + previous frame) +
  output projection + residual + LN3 + GEGLU FF + residual (in place).
  Reshard: strided-AP DMA staging + AllToAll (bf16); the frame-major ->
  seq-major scatter rides inside the unstage DMA access patterns.
  Phase D (spatial-parallel): LNt + temporal attention over the 16 frames of
  each spatial token (block-diagonal masked scores) + out proj + residual.

This environment prices execution per *instruction* (~65us/matmul,
~40us/DVE/ACT, ~20us/DMA, collectives/gpsimd cheap), so everything
minimizes instruction count: N=512 matmuls (PSUM bank limit), multi-bank
PSUM tiles evacuated with single wide ACT/DVE ops, bulk DMAs, no PE
transposes (layout changes ride on DMA access patterns), LN stats via
ones-matmuls. Residuals ride in bf16 (tolerance 2e-2).
"""

import math

import numpy as np
import ml_dtypes

import concourse.bass as bass
import concourse.bacc as bacc
import concourse.tile as tile
from concourse import mybir
from concourse.bass_utils import run_bass_kernel_spmd

BF16 = mybir.dt.bfloat16
F32 = mybir.dt.float32
AF = mybir.ActivationFunctionType
OP = mybir.AluOpType

H, DH, C, F, D = 8, 80, 640, 16, 1024
INNER = 4 * C            # 2560
NI = INNER // 128        # 20
NCORES = 8
KT = C // 128            # 5 feature k-tiles
NQ = 2 * D               # 2048 own tokens / core
NKV = 3 * D              # 3072 kv-context tokens / core
NU = 4 * D               # 4096 union tokens / core
EPS = 1e-5
RG = [list(range(NCORES))]

_BUILD_CACHE = {}


def _build(reps=1, use_cc=True):
    key = (reps, use_cc)
    if key in _BUILD_CACHE:
        return _BUILD_CACHE[key]
    nc = bacc.Bacc("TRN2", target_bir_lowering=False, debug=False,
                   num_devices=NCORES)

    def din(name, shape, dt):
        return nc.dram_tensor(name, shape, dt, kind="ExternalInput").ap()

    xub_d = din("xub", [KT, 128, NU], BF16)     # union x, feature-major bf16
    wq_d = din("wq", [KT, 128, C], BF16)
    wk_d = din("wk", [KT, 128, C], BF16)
    wv_d = din("wv", [KT, 128, C], BF16)
    wo1_d = din("wo1", [H, 81, C], BF16)        # per-head o-proj (row 80 = 0)
    wf1_d = din("wf1", [KT, 128, 2 * INNER], BF16)
    wf2_d = din("wf2", [NI, 128, C], BF16)
    wqt_d = din("wqt", [KT, 128, C], BF16)
    wkt_d = din("wkt", [KT, 128, C], BF16)
    wvt_d = din("wvt", [KT, 128, C], BF16)
    wot_d = din("wot", [H, 81, C], BF16)
    bq_d = din("bq", [C], F32)
    bk_d = din("bk", [C], F32)
    bv_d = din("bv", [C], F32)
    bo1_d = din("bo1", [C], F32)
    bf1_d = din("bf1", [2 * INNER], F32)
    bf2_d = din("bf2", [C], F32)
    bqt_d = din("bqt", [C], F32)
    bkt_d = din("bkt", [C], F32)
    bvt_d = din("bvt", [C], F32)
    bot_d = din("bot", [C], F32)
    mask_d = din("mask", [128, NQ], BF16)       # block-diag mask, tiled x16
    out_d = nc.dram_tensor("out", [KT, 128, NQ], F32,
                           kind="ExternalOutput").ap()

    with tile.TileContext(nc) as tc:
        with tc.tile_pool(name="const", bufs=1) as constp, \
             tc.tile_pool(name="dram", bufs=1, space="DRAM") as dramp:

            # ---------------- global constants -------------
            ones_b = constp.tile([128, 128], BF16)
            nc.vector.memset(ones_b, 1.0)
            epst = constp.tile([128, 1], F32)
            nc.vector.memset(epst, EPS)

            def bias_tile(b_d, ncols):
                t = constp.tile([128, ncols], F32, tag=f"b_{b_d.tensor.name}",
                                name=f"b_{b_d.tensor.name}")
                nc.sync.dma_start(out=t, in_=b_d.rearrange("(m p) -> p m", p=128))
                return t

            bq_t = bias_tile(bq_d, KT)
            bk_t = bias_tile(bk_d, KT)
            bo1_t = bias_tile(bo1_d, KT)
            bf1_t = bias_tile(bf1_d, 2 * INNER // 128)
            bf2_t = bias_tile(bf2_d, KT)
            bqt_t = bias_tile(bqt_d, KT)
            bkt_t = bias_tile(bkt_d, KT)
            bot_t = bias_tile(bot_d, KT)

            def bcast_bias(b_d):
                t = constp.tile([128, C], F32, tag=f"bb_{b_d.tensor.name}",
                                name=f"bb_{b_d.tensor.name}")
                nc.sync.dma_start(out=t, in_=bass.AP(
                    tensor=b_d.tensor, offset=b_d.offset,
                    ap=[[0, 128]] + list(b_d.ap)))
                return t

            bvbc = bcast_bias(bv_d)
            bvtbc = bcast_bias(bvt_d)

            def ln_stats(pool, psp, wkp, xs, ntok, tag):
                """xs: list of KT SBUF views [128, ntok] bf16.
                Returns (M, R) bf16 [128, ntok] replicated across partitions."""
                nbl = ntok // 2048
                ps_s = psp.tile([128, 2048], F32, tag=f"pss_{tag}")
                ps_q = psp.tile([128, 2048], F32, tag=f"psq_{tag}")
                M = pool.tile([128, ntok], BF16, tag=f"M_{tag}", name=f"M_{tag}")
                S = wkp.tile([128, ntok], F32, tag=f"S_{tag}", name=f"S_{tag}")
                for bl in range(nbl):
                    bsl = slice(bl * 2048, (bl + 1) * 2048)
                    for kt in range(KT):
                        sq = wkp.tile([128, 2048], BF16, tag=f"sq_{tag}",
                                      name=f"sq_{tag}", bufs=2)
                        nc.vector.tensor_mul(sq, xs[kt][:, bsl],
                                             xs[kt][:, bsl])
                        for ch in range(4):
                            sl = slice(ch * 512, (ch + 1) * 512)
                            gsl = slice(bl * 2048 + ch * 512,
                                        bl * 2048 + (ch + 1) * 512)
                            nc.tensor.matmul(ps_s[:, sl], ones_b,
                                             xs[kt][:, gsl],
                                             start=(kt == 0),
                                             stop=(kt == KT - 1))
                            nc.tensor.matmul(ps_q[:, sl], ones_b,
                                             sq[:, sl],
                                             start=(kt == 0),
                                             stop=(kt == KT - 1))
                    nc.scalar.activation(out=M[:, bsl], in_=ps_s,
                                         func=AF.Identity, scale=1.0 / C)
                    nc.scalar.activation(out=S[:, bsl], in_=ps_q,
                                         func=AF.Identity, scale=1.0 / C)
                    msq = wkp.tile([128, 2048], BF16, tag=f"sq_{tag}",
                                   name=f"msq_{tag}", bufs=2)
                    nc.vector.tensor_mul(msq, M[:, bsl], M[:, bsl])
                    nc.vector.tensor_sub(S[:, bsl], S[:, bsl], msq)
                nc.scalar.activation(out=S, in_=S, func=AF.Sqrt, bias=epst)
                R = pool.tile([128, ntok], BF16, tag=f"R_{tag}", name=f"R_{tag}")
                with nc.allow_low_precision(reason="rstd bf16 ok"):
                    nc.vector.reciprocal(out=R, in_=S)
                return M, R

            def emit(it):
                kh_d = dramp.tile([C, NKV], BF16, tag="kh_d", name="kh_d")
                qh_d = dramp.tile([C, NQ], BF16, tag="qh_d", name="qh_d")
                a2a_in = dramp.tile([NCORES, C, 256], BF16, tag="a2a_in",
                                    name="a2a_in")
                a2a_out = dramp.tile([NCORES, C, 256], BF16, tag="a2a_out",
                                     name="a2a_out")

                with tc.tile_pool(name="pH", bufs=1) as pH:
                    h1 = pH.tile([128, KT, NQ], BF16, tag="h1")
                    with tc.tile_pool(name="pAB", bufs=1) as pAB:
                        xu = pAB.tile([128, KT, NU], BF16, tag="xu")
                        nc.sync.dma_start(
                            out=xu, in_=xub_d.rearrange("k p c -> p k c"))
                        v3 = pAB.tile([128, NKV // 128, H, 81], BF16,
                                      tag="v3")

                        # ===== Phase A: LN1 + q/k/v projections =====
                        with tc.tile_pool(name="pA", bufs=1) as pA, \
                             tc.tile_pool(name="pAw", bufs=1) as pAw:
                            with tc.tile_pool(name="psS", bufs=1,
                                              space="PSUM") as psS:
                                xus = [xu[:, kt, :] for kt in range(KT)]
                                M1, R1 = ln_stats(pA, psS, pAw, xus, NU,
                                                  "ln1")
                            xh = []
                            for kt in range(KT):
                                t = pA.tile([128, NU], BF16, tag=f"xh{kt}",
                                            name=f"xh{kt}")
                                nc.vector.tensor_sub(t, xu[:, kt, :], M1)
                                nc.vector.tensor_mul(t, t, R1)
                                xh.append(t)

                            wq_t = pA.tile([128, KT, C], BF16, tag="wq")
                            nc.sync.dma_start(
                                out=wq_t, in_=wq_d.rearrange("k p c -> p k c"))
                            wk_t = pA.tile([128, KT, C], BF16, tag="wk")
                            nc.sync.dma_start(
                                out=wk_t, in_=wk_d.rearrange("k p c -> p k c"))
                            wv_t = pA.tile([128, KT, C], BF16, tag="wv")
                            nc.sync.dma_start(
                                out=wv_t, in_=wv_d.rearrange("k p c -> p k c"))

                            with tc.tile_pool(name="psPq", bufs=2,
                                              space="PSUM") as psPq:
                                for m in range(KT):
                                    msl = slice(m * 128, (m + 1) * 128)
                                    pq = psPq.tile([128, NQ], F32, tag="pq")
                                    for ch in range(4):
                                        sl = slice(ch * 512, (ch + 1) * 512)
                                        usl = slice(NU - NQ + ch * 512,
                                                    NU - NQ + (ch + 1) * 512)
                                        for kt in range(KT):
                                            nc.tensor.matmul(
                                                pq[:, sl], wq_t[:, kt, msl],
                                                xh[kt][:, usl],
                                                start=(kt == 0),
                                                stop=(kt == KT - 1))
                                    qe = pAw.tile([128, NQ], BF16, tag="qe")
                                    nc.scalar.activation(
                                        out=qe, in_=pq, func=AF.Identity,
                                        bias=bq_t[:, m:m + 1])
                                    nc.sync.dma_start(out=qh_d[msl, :],
                                                      in_=qe)
                            with tc.tile_pool(name="psPk", bufs=1,
                                              space="PSUM") as psPk:
                                for m in range(KT):
                                    msl = slice(m * 128, (m + 1) * 128)
                                    pk = psPk.tile([128, NKV], F32, tag="pk")
                                    for ch in range(6):
                                        sl = slice(ch * 512, (ch + 1) * 512)
                                        for kt in range(KT):
                                            nc.tensor.matmul(
                                                pk[:, sl], wk_t[:, kt, msl],
                                                xh[kt][:, sl],
                                                start=(kt == 0),
                                                stop=(kt == KT - 1))
                                    ke = pAw.tile([128, NKV], BF16, tag="ke")
                                    nc.scalar.activation(
                                        out=ke, in_=pk, func=AF.Identity,
                                        bias=bk_t[:, m:m + 1])
                                    nc.sync.dma_start(out=kh_d[msl, :],
                                                      in_=ke)
                            with tc.tile_pool(name="psPv", bufs=2,
                                              space="PSUM") as psPv:
                                nc.vector.memset(v3[:, :, :, 80], 1.0)
                                for tt in range(NKV // 128):
                                    tsl = slice(tt * 128, (tt + 1) * 128)
                                    # [128, 2, 512]: half hf in bank hf,
                                    # payload in cols 0:320 (bank-aligned)
                                    pv = psPv.tile([128, 1024], F32, tag="pv")
                                    for hf in range(2):
                                        csl = slice(hf * 320, (hf + 1) * 320)
                                        for kt in range(KT):
                                            nc.tensor.matmul(
                                                pv[:, hf * 512:
                                                   hf * 512 + 320],
                                                xh[kt][:, tsl],
                                                wv_t[:, kt, csl],
                                                start=(kt == 0),
                                                stop=(kt == KT - 1))
                                    pvv = (pv.rearrange("p (b c) -> p b c",
                                                        b=2)[:, :, 0:320]
                                           .rearrange("p b (h c) -> p b h c",
                                                      c=DH))
                                    nc.vector.tensor_add(
                                        v3[:, tt, :, 0:80]
                                        .rearrange("p (b h) c -> p b h c",
                                                   b=2),
                                        pvv,
                                        bvbc.rearrange("p (h c) -> p h c",
                                                       c=DH)
                                        .rearrange("p (b h) c -> p b h c",
                                                   b=2))

                        # ===== Phase B: sparse-causal attention =====
                        with tc.tile_pool(name="pB", bufs=1) as pB, \
                             tc.tile_pool(name="pBw", bufs=1) as pBw, \
                             tc.tile_pool(name="pBd", bufs=8) as pBd, \
                             tc.tile_pool(name="psB", bufs=1,
                                          space="PSUM") as psB:
                            wo1_t = []
                            for h in range(H):
                                t = pB.tile([81, C], BF16, tag=f"wo1_{h}",
                                            name=f"wo1_{h}")
                                nc.sync.dma_start(out=t, in_=wo1_d[h])
                                wo1_t.append(t)
                            khs0 = []
                            for h in range(H):
                                t = pB.tile([80, D], BF16, tag=f"khs0_{h}",
                                            name=f"khs0_{h}")
                                nc.sync.dma_start(
                                    out=t, in_=kh_d[h * DH:(h + 1) * DH, 0:D])
                                khs0.append(t)
                            for fi in range(2):
                                attD = []
                                for h in range(H):
                                    hrow = slice(h * DH, (h + 1) * DH)
                                    khsp = pBw.tile([80, D], BF16, tag="khsp",
                                                    bufs=2)
                                    nc.sync.dma_start(
                                        out=khsp,
                                        in_=kh_d[hrow,
                                                 (1 + fi) * D:(2 + fi) * D])
                                    qs = pBw.tile([80, D], BF16, tag="qs",
                                                  bufs=2)
                                    nc.sync.dma_start(
                                        out=qs,
                                        in_=qh_d[hrow, fi * D:(fi + 1) * D])
                                    pvp = psB.tile([81, D], F32, tag="ppv")
                                    # 8 waves of 2 key-tiles x full queries
                                    for w in range(8):
                                        pP = psB.tile([128, 2048], F32,
                                                      tag="pP")
                                        for t2 in range(2):
                                            t = w * 2 + t2
                                            lhs = (khs0[h][:, (t % 8) * 128:
                                                           (t % 8 + 1) * 128]
                                                   if t < 8 else
                                                   khsp[:, (t - 8) * 128:
                                                        (t - 7) * 128])
                                            for qc in range(2):
                                                qsl = slice(qc * 512,
                                                            (qc + 1) * 512)
                                                nc.tensor.matmul(
                                                    pP[:, t2 * 1024 + qc * 512:
                                                       t2 * 1024 +
                                                       (qc + 1) * 512],
                                                    lhs, qs[:, qsl],
                                                    start=True, stop=True)
                                        Pw = pBw.tile([128, 2048], BF16,
                                                      tag="Pw", bufs=2)
                                        nc.scalar.activation(out=Pw, in_=pP,
                                                             func=AF.Exp)
                                        for t2 in range(2):
                                            t = w * 2 + t2
                                            tt = (t if t < 8
                                                  else (1 + fi) * 8 + (t - 8))
                                            for qc in range(2):
                                                qsl = slice(qc * 512,
                                                            (qc + 1) * 512)
                                                nc.tensor.matmul(
                                                    pvp[:, qsl],
                                                    v3[:, tt, h, :],
                                                    Pw[:, t2 * 1024 + qc * 512:
                                                       t2 * 1024 +
                                                       (qc + 1) * 512],
                                                    start=(t == 0),
                                                    stop=(t == 15))
                                    attP = pBw.tile([81, D], BF16, tag="attP")
                                    nc.vector.tensor_copy(attP, pvp)
                                    dnm = pBw.tile([1, D], BF16, tag="dnm")
                                    nc.sync.dma_start(out=dnm,
                                                      in_=attP[80:81, :])
                                    dnr = pBw.tile([1, D], F32, tag="dnr")
                                    nc.vector.reciprocal(out=dnr, in_=dnm)
                                    aB = pBw.tile([80, D], F32, tag="aB")
                                    nc.gpsimd.partition_broadcast(
                                        aB, dnr[0:1, :], channels=80)
                                    aD = pBd.tile([81, D], BF16, tag="attD",
                                                  name="attD")
                                    nc.vector.memset(aD, 0.0)
                                    nc.vector.tensor_mul(aD[0:80, :],
                                                         attP[0:80, :], aB)
                                    attD.append(aD)
                                for m in range(KT):
                                    po = psB.tile([128, D], F32, tag="po",
                                                  bufs=1)
                                    for qc in range(2):
                                        qsl = slice(qc * 512, (qc + 1) * 512)
                                        for h in range(H):
                                            nc.tensor.matmul(
                                                po[:, qsl],
                                                wo1_t[h][:, m * 128:
                                                         (m + 1) * 128],
                                                attD[h][:, qsl],
                                                start=(h == 0),
                                                stop=(h == H - 1))
                                    nc.vector.scalar_tensor_tensor(
                                        out=h1[:, m, fi * D:(fi + 1) * D],
                                        in0=po, scalar=bo1_t[:, m:m + 1],
                                        in1=xu[:, m, NU - NQ + fi * D:
                                               NU - NQ + (fi + 1) * D],
                                        op0=OP.add, op1=OP.add)

                    # ===== Phase C: LN3 + GEGLU FF (h2 written into h1) =====
                    with tc.tile_pool(name="pC", bufs=1) as pC, \
                         tc.tile_pool(name="pCw", bufs=2) as pCw:
                        with tc.tile_pool(name="psS3", bufs=1,
                                          space="PSUM") as psS3:
                            h1s = [h1[:, kt, :] for kt in range(KT)]
                            M3, R3 = ln_stats(pC, psS3, pCw, h1s, NQ, "ln3")
                        xh3 = []
                        for kt in range(KT):
                            t = pC.tile([128, NQ], BF16, tag=f"xh3_{kt}",
                                        name=f"xh3_{kt}")
                            nc.vector.tensor_sub(t, h1[:, kt, :], M3)
                            nc.vector.tensor_mul(t, t, R3)
                            xh3.append(t)
                        wf1_t = pC.tile([128, KT, 2 * INNER], BF16, tag="wf1")
                        nc.sync.dma_start(
                            out=wf1_t, in_=wf1_d.rearrange("k p c -> p k c"))
                        wf2_t = pC.tile([128, NI, C], BF16, tag="wf2")
                        nc.sync.dma_start(
                            out=wf2_t, in_=wf2_d.rearrange("k p c -> p k c"))
                        with tc.tile_pool(name="psF", bufs=1,
                                          space="PSUM") as psF:
                            for half in range(2):
                                sl = slice(half * 1024, (half + 1) * 1024)
                                ffin = pC.tile([128, NI, 1024], BF16,
                                               tag="ffin")
                                for j in range(NI):
                                    psa = psF.tile([128, 1024], F32,
                                                   tag="psa")
                                    psg = psF.tile([128, 1024], F32,
                                                   tag="psg")
                                    for qc in range(2):
                                        qsl = slice(half * 1024 + qc * 512,
                                                    half * 1024 +
                                                    (qc + 1) * 512)
                                        osl = slice(qc * 512, (qc + 1) * 512)
                                        for kt in range(KT):
                                            nc.tensor.matmul(
                                                psa[:, osl],
                                                wf1_t[:, kt, j * 128:
                                                      (j + 1) * 128],
                                                xh3[kt][:, qsl],
                                                start=(kt == 0),
                                                stop=(kt == KT - 1))
                                        for kt in range(KT):
                                            nc.tensor.matmul(
                                                psg[:, osl],
                                                wf1_t[:, kt, INNER + j * 128:
                                                      INNER + (j + 1) * 128],
                                                xh3[kt][:, qsl],
                                                start=(kt == 0),
                                                stop=(kt == KT - 1))
                                    gg = pCw.tile([128, 1024], BF16, tag="gg")
                                    nc.scalar.activation(
                                        out=gg, in_=psg, func=AF.Gelu,
                                        bias=bf1_t[:, NI + j:NI + j + 1])
                                    nc.vector.scalar_tensor_tensor(
                                        out=ffin[:, j, :], in0=psa,
                                        scalar=bf1_t[:, j:j + 1], in1=gg,
                                        op0=OP.add, op1=OP.mult)
                                for m in range(KT):
                                    pf = psF.tile([128, 1024], F32, tag="pf")
                                    for qc in range(2):
                                        osl = slice(qc * 512, (qc + 1) * 512)
                                        for j in range(NI):
                                            nc.tensor.matmul(
                                                pf[:, osl],
                                                wf2_t[:, j, m * 128:
                                                      (m + 1) * 128],
                                                ffin[:, j, osl],
                                                start=(j == 0),
                                                stop=(j == NI - 1))
                                    nc.vector.scalar_tensor_tensor(
                                        out=h1[:, m, sl], in0=pf,
                                        scalar=bf2_t[:, m:m + 1],
                                        in1=h1[:, m, sl],
                                        op0=OP.add, op1=OP.add)

                    # ===== reshard: (frames) -> (spatial slice) =====
                    for kt in range(KT):
                        hv = h1[:, kt, :].rearrange(
                            "p (fl j s) -> p j fl s", fl=2, j=NCORES)
                        for j in range(NCORES):
                            nc.sync.dma_start(
                                out=a2a_in[j, kt * 128:(kt + 1) * 128, :],
                                in_=hv[:, j])
                if use_cc:
                    nc.gpsimd.collective_compute(
                        "AllToAll", OP.bypass, replica_groups=RG,
                        ins=[a2a_in[:, :, :]], outs=[a2a_out[:, :, :]])
                else:
                    nc.sync.dma_start(out=a2a_out[:, :, :],
                                      in_=a2a_in[:, :, :])

                # ===== Phase D: temporal attention (seq-major) =====
                with tc.tile_pool(name="pD", bufs=1) as pD, \
                     tc.tile_pool(name="pDw", bufs=1) as pDw:
                    mask_t = pD.tile([128, NQ], BF16, tag="mask")
                    nc.sync.dma_start(out=mask_t, in_=mask_d[:, :])
                    ht = pD.tile([128, KT, NQ], BF16, tag="ht")
                    for kt in range(KT):
                        hfm = pDw.tile([128, NQ], BF16, tag="hfm", bufs=1)
                        for j in range(NCORES):
                            nc.sync.dma_start(
                                out=hfm[:, j * 256:(j + 1) * 256],
                                in_=a2a_out[j, kt * 128:(kt + 1) * 128, :])
                        # frame-major (f*128+s) -> seq-major (s*16+f)
                        nc.vector.tensor_copy(
                            ht[:, kt, :].rearrange("p (s f) -> p f s", f=F),
                            hfm.rearrange("p (f s) -> p f s", f=F))
                    with tc.tile_pool(name="psSt", bufs=1,
                                      space="PSUM") as psSt:
                        hts = [ht[:, kt, :] for kt in range(KT)]
                        Mt, Rt = ln_stats(pD, psSt, pDw, hts, NQ, "lnt")
                    xht = []
                    for kt in range(KT):
                        t = pD.tile([128, NQ], BF16, tag=f"xht{kt}",
                                    name=f"xht{kt}")
                        nc.vector.tensor_sub(t, ht[:, kt, :], Mt)
                        nc.vector.tensor_mul(t, t, Rt)
                        xht.append(t)

                    wvt_t = pD.tile([128, KT, C], BF16, tag="wvt")
                    nc.sync.dma_start(out=wvt_t,
                                      in_=wvt_d.rearrange("k p c -> p k c"))
                    wqt_t = pD.tile([128, KT, C], BF16, tag="wqt")
                    nc.sync.dma_start(out=wqt_t,
                                      in_=wqt_d.rearrange("k p c -> p k c"))
                    wkt_t = pD.tile([128, KT, C], BF16, tag="wkt")
                    nc.sync.dma_start(out=wkt_t,
                                      in_=wkt_d.rearrange("k p c -> p k c"))
                    wot_t = []
                    for h in range(H):
                        t = pD.tile([81, C], BF16, tag=f"wot_{h}",
                                    name=f"wot_{h}")
                        nc.sync.dma_start(out=t, in_=wot_d[h])
                        wot_t.append(t)

                    for half in range(2):
                        hsl = slice(half * 1024, (half + 1) * 1024)
                        with tc.tile_pool(name="pDh", bufs=1) as pDh:
                            vt3 = pDh.tile([128, 8, H, 81], BF16, tag="vt3")
                            qth, kth = [], []
                            with tc.tile_pool(name="psD1", bufs=1,
                                              space="PSUM") as psD1:
                                nc.vector.memset(vt3[:, :, :, 80], 1.0)
                                for st in range(8):
                                    tsl = slice(half * 1024 + st * 128,
                                                half * 1024 + (st + 1) * 128)
                                    pvt = psD1.tile([128, 1024], F32,
                                                    tag="pvt", bufs=2)
                                    for hf in range(2):
                                        csl = slice(hf * 320, (hf + 1) * 320)
                                        for kt in range(KT):
                                            nc.tensor.matmul(
                                                pvt[:, hf * 512:
                                                    hf * 512 + 320],
                                                xht[kt][:, tsl],
                                                wvt_t[:, kt, csl],
                                                start=(kt == 0),
                                                stop=(kt == KT - 1))
                                    pvv = (pvt.rearrange("p (b c) -> p b c",
                                                         b=2)[:, :, 0:320]
                                           .rearrange("p b (h c) -> p b h c",
                                                      c=DH))
                                    nc.vector.tensor_add(
                                        vt3[:, st, :, 0:80]
                                        .rearrange("p (b h) c -> p b h c",
                                                   b=2),
                                        pvv,
                                        bvtbc.rearrange("p (h c) -> p h c",
                                                        c=DH)
                                        .rearrange("p (b h) c -> p b h c",
                                                   b=2))
                                qkm = {}
                                for nm, wt, bt in (("q", wqt_t, bqt_t),
                                                   ("k", wkt_t, bkt_t)):
                                    for m in range(KT):
                                        pqk = psD1.tile([128, 1024], F32,
                                                        tag="pqk", bufs=2)
                                        for qc in range(2):
                                            osl = slice(qc * 512,
                                                        (qc + 1) * 512)
                                            gsl = slice(half * 1024 +
                                                        qc * 512,
                                                        half * 1024 +
                                                        (qc + 1) * 512)
                                            for kt in range(KT):
                                                nc.tensor.matmul(
                                                    pqk[:, osl],
                                                    wt[:, kt, m * 128:
                                                       (m + 1) * 128],
                                                    xht[kt][:, gsl],
                                                    start=(kt == 0),
                                                    stop=(kt == KT - 1))
                                        qt = pDh.tile([128, 1024], BF16,
                                                      tag=f"qkm_{nm}{m}",
                                                      name=f"qkm_{nm}{m}")
                                        nc.scalar.activation(
                                            out=qt, in_=pqk, func=AF.Identity,
                                            bias=bt[:, m:m + 1])
                                        qkm[(nm, m)] = qt
                                for nm, dst in (("q", qth), ("k", kth)):
                                    for h in range(H):
                                        t = pDh.tile([80, 1024], BF16,
                                                     tag=f"{nm}th{h}",
                                                     name=f"{nm}th{h}")
                                        r0 = h * DH
                                        off = 0
                                        while off < DH:
                                            m = (r0 + off) // 128
                                            mo = (r0 + off) % 128
                                            ln = min(DH - off, 128 - mo)
                                            nc.sync.dma_start(
                                                out=t[off:off + ln, :],
                                                in_=qkm[(nm, m)]
                                                [mo:mo + ln, :])
                                            off += ln
                                        dst.append(t)
                            attDt = []
                            with tc.tile_pool(name="psD2", bufs=1,
                                              space="PSUM") as psD2:
                                for h in range(H):
                                    psc = psD2.tile([128, 1024], F32,
                                                    tag="psc")
                                    for st in range(8):
                                        tsl = slice(st * 128, (st + 1) * 128)
                                        nc.tensor.matmul(psc[:, tsl],
                                                         kth[h][:, tsl],
                                                         qth[h][:, tsl],
                                                         start=True,
                                                         stop=True)
                                    Pm = pDw.tile([128, 1024], BF16,
                                                  tag="Pm")
                                    nc.scalar.activation(out=Pm, in_=psc,
                                                         func=AF.Exp)
                                    nc.vector.tensor_mul(Pm, Pm,
                                                         mask_t[:, hsl])
                                    ppvt = psD2.tile([81, 1024], F32,
                                                     tag="ppvt")
                                    for st in range(8):
                                        tsl = slice(st * 128, (st + 1) * 128)
                                        nc.tensor.matmul(ppvt[:, tsl],
                                                         vt3[:, st, h, :],
                                                         Pm[:, tsl],
                                                         start=True,
                                                         stop=True)
                                    attP = pDw.tile([81, 1024], BF16,
                                                    tag="attPt")
                                    nc.vector.tensor_copy(attP, ppvt)
                                    dnm = pDw.tile([1, 1024], BF16,
                                                   tag="dnmt")
                                    nc.sync.dma_start(out=dnm,
                                                      in_=attP[80:81, :])
                                    dnr = pDw.tile([1, 1024], F32,
                                                   tag="dnrt")
                                    nc.vector.reciprocal(out=dnr, in_=dnm)
                                    aB = pDw.tile([80, 1024], F32,
                                                  tag="aBt")
                                    nc.gpsimd.partition_broadcast(
                                        aB, dnr[0:1, :], channels=80)
                                    aD = pDh.tile([81, 1024], BF16,
                                                  tag=f"attDt{h}",
                                                  name=f"attDt{h}")
                                    nc.vector.memset(aD, 0.0)
                                    nc.vector.tensor_mul(aD[0:80, :],
                                                         attP[0:80, :], aB)
                                    attDt.append(aD)
                                for m in range(KT):
                                    pot = psD2.tile([128, 1024], F32,
                                                    tag="pot", bufs=2)
                                    for qc in range(2):
                                        osl = slice(qc * 512, (qc + 1) * 512)
                                        for h in range(H):
                                            nc.tensor.matmul(
                                                pot[:, osl],
                                                wot_t[h][:, m * 128:
                                                         (m + 1) * 128],
                                                attDt[h][:, osl],
                                                start=(h == 0),
                                                stop=(h == H - 1))
                                    oe = pDw.tile([128, 1024], F32, tag="oe",
                                                  bufs=2)
                                    nc.vector.scalar_tensor_tensor(
                                        out=oe, in0=pot,
                                        scalar=bot_t[:, m:m + 1],
                                        in1=ht[:, m, hsl],
                                        op0=OP.add, op1=OP.add)
                                    nc.sync.dma_start(out=out_d[m, :, hsl],
                                                      in_=oe)

            for it in range(reps):
                emit(it)

    nc.compile()
    _BUILD_CACHE[key] = nc
    return nc


def _prep_inputs(hidden_states, ln1_g, ln1_b, q1w, k1w, v1w, o1w, o1b,
                 ln3_g, ln3_b, ff_w1, ff_b1, ff_w2, ff_b2,
                 lnt_g, lnt_b, qtw, ktw, vtw, otw, otb):
    """Host-side weight folding + per-core input shards."""
    bf = ml_dtypes.bfloat16
    sc = 1.0 / math.sqrt(DH)

    def fold(g, b, w):
        return g[:, None] * w, b @ w

    wq, bq = fold(ln1_g, ln1_b, q1w)
    wq, bq = wq * sc, bq * sc
    wk, bk = fold(ln1_g, ln1_b, k1w)
    wv, bv = fold(ln1_g, ln1_b, v1w)
    # o1w quirk: channel-major interleave -> padded per-head [81, C] with the
    # original row dh*H + h at padded position (h, dh); row 80 is zero
    # (multiplies the softmax-denominator row).
    wo1 = np.zeros((H, 81, C), np.float32)
    idx_dh = np.arange(DH)
    for h in range(H):
        wo1[h, 0:DH, :] = o1w[idx_dh * H + h, :]
    wf1, bf1 = fold(ln3_g, ln3_b, ff_w1)
    bf1 = bf1 + ff_b1
    wqt, bqt = fold(lnt_g, lnt_b, qtw)
    wqt, bqt = wqt * sc, bqt * sc
    wkt, bkt = fold(lnt_g, lnt_b, ktw)
    wvt, bvt = fold(lnt_g, lnt_b, vtw)
    wot = np.zeros((H, 81, C), np.float32)
    for h in range(H):
        wot[h, 0:DH, :] = otw[h * DH + idx_dh, :]

    # 8 sequences per 128-token tile; block-diag of 8 16x16 blocks,
    # tiled across the full 2048 columns (16 seq-tiles).
    mask = np.kron(np.eye(8, dtype=np.float32), np.ones((F, F), np.float32))
    mask_full = np.tile(mask, (1, F))

    def c(a, dt=bf):
        return np.ascontiguousarray(np.asarray(a, np.float32).astype(dt))

    shared = dict(
        wq=c(wq.reshape(KT, 128, C)), wk=c(wk.reshape(KT, 128, C)),
        wv=c(wv.reshape(KT, 128, C)), wo1=c(wo1),
        wf1=c(wf1.reshape(KT, 128, 2 * INNER)),
        wf2=c(np.asarray(ff_w2, np.float32).reshape(INNER // 128, 128, C)),
        wqt=c(wqt.reshape(KT, 128, C)), wkt=c(wkt.reshape(KT, 128, C)),
        wvt=c(wvt.reshape(KT, 128, C)), wot=c(wot),
        bq=c(bq, np.float32), bk=c(bk, np.float32), bv=c(bv, np.float32),
        bo1=c(o1b, np.float32), bf1=c(bf1, np.float32),
        bf2=c(ff_b2, np.float32),
        bqt=c(bqt, np.float32), bkt=c(bkt, np.float32),
        bvt=c(bvt, np.float32), bot=c(otb, np.float32),
        mask=c(mask_full),
    )

    hs = np.asarray(hidden_states, np.float32)   # [BF, D, C]
    in_maps = []
    for i in range(NCORES):
        fa, fb = 2 * i, 2 * i + 1
        fprev = max(2 * i - 1, 0)
        frames = [0, fprev, fa, fb]
        xu = hs[frames].reshape(NU, C).T          # [C, NU] feature-major
        m = dict(shared)
        m["xub"] = np.ascontiguousarray(xu.astype(bf).reshape(KT, 128, NU))
        in_maps.append(m)
    return in_maps


def kernel(**inputs):
    video_length = int(np.asarray(inputs.pop("video_length")))
    assert video_length == F, f"kernel hardcodes F={F}, got {video_length}"
    in_maps = _prep_inputs(**{k: np.asarray(v) for k, v in inputs.items()})
    nc = _build(reps=1)
    res = run_bass_kernel_spmd(nc, in_maps, list(range(NCORES)))
    out = np.empty((F, D, C), np.float32)
    for i in range(NCORES):
        r = res.results[i]["out"].reshape(C, D // NCORES, F)   # [c, s, f]
        out[:, i * (D // NCORES):(i + 1) * (D // NCORES), :] = r.transpose(2, 1, 0)
    return out
